# revision 15
# baseline (speedup 1.0000x reference)
"""NeRF-style render kernel for TRN2 (8 NeuronCores, data-parallel over rays).

Self-contained: hardcodes all shapes. Coarse proposal MLP runs in fp32
(resampling is precision-critical), fine MLP in float32r.
"""
import os
import sys

sys.path.insert(0, '/opt/trn_rl_repo')
import numpy as np
import concourse.bass as bass
import concourse.bacc as bacc
import concourse.tile as tile
import concourse.mybir as mybir
from concourse.bass_utils import run_bass_kernel_spmd

F32 = mybir.dt.float32
F32R = mybir.dt.float32r
AF = mybir.ActivationFunctionType
OP = mybir.AluOpType

NCORES = 8
R = 128          # rays per core
S = 128          # samples per pass
CHUNK_RAYS = 16  # rays per chunk
NCHUNK = R // CHUNK_RAYS          # 8
CN = CHUNK_RAYS * S               # 2048 cols per chunk
TILE_N = 512                      # matmul moving size
NTILE = CN // TILE_N              # 4 point-tiles per chunk

MAGIC = np.float32(12582912.0)    # 1.5 * 2^23 (round-to-int trick)
INV2PI = np.float32(1.0 / (2.0 * np.pi))
C1 = np.float32(6.28125)          # 2*pi split, k*C1 exact for k < 2^13
C2 = np.float32(2.0 * np.pi - 6.28125)

BUILD_STAGE = int(os.environ.get("KERNEL_STAGE", "3"))
DEBUG_OUT = os.environ.get("KERNEL_DEBUG", "0") == "1"


# ---------------------------------------------------------------- host prep
def _posenc_rows(nf, span=None, minp=None):
    """A3 [6*nf,3] / const [6*nf] for rows f-major: per f: 3 sin, 3 cos."""
    rows = 6 * nf
    A3 = np.zeros((rows, 3), np.float64)
    ph = np.zeros((rows,), np.float64)
    for f in range(nf):
        for k in range(6):
            r = 6 * f + k
            d = k % 3
            sc = 2.0 ** f
            if span is not None:
                A3[r, d] = sc / span[d]
                ph[r] = -sc * minp[d] / span[d]
            else:
                A3[r, d] = sc
            if k >= 3:
                ph[r] += np.pi / 2.0
    return A3, ph


def host_prep(inp):
    c = {}
    f32 = np.float32

    # coarse posenc: selector*2^f matrix [3,60] + phase col [60,1]
    A3s, phs = _posenc_rows(10)
    c['cA3selT'] = A3s.T.astype(f32).copy()                      # [3,60]
    c['cphasecol'] = np.asarray(phs, np.float64).astype(f32).reshape(-1, 1)

    # fine posenc rows: [sinx60, sinapp36, xyz3, appx3]
    minp = inp['min_point'].astype(np.float64)
    span = (inp['max_point'] - inp['min_point']).astype(np.float64)
    A3a, pha = _posenc_rows(6, span=span, minp=minp)
    pad4 = np.zeros((4, 3))
    fA3 = np.concatenate([A3s, pad4, A3a, np.eye(3), np.diag(1.0 / span)], 0)
    fph = np.concatenate([phs, np.zeros(4), pha, np.zeros(3), -minp / span], 0)
    c['fA3T'] = fA3.T.astype(f32).copy()                         # [3,106]
    c['fA4T'] = np.concatenate([fA3, fph[:, None]], 1).T.astype(f32).copy()

    # per-ray enc matrices (lhsT)
    Ad = np.zeros((24, 4), np.float64)
    for f in range(4):
        for k in range(6):
            r = 6 * f + k
            Ad[r, k % 3] = 2.0 ** f
            if k >= 3:
                Ad[r, 3] = np.pi / 2.0
    c['AdT'] = Ad.T.astype(f32).copy()                           # [4,24]
    At = np.zeros((12, 2), np.float64)
    for f in range(6):
        At[2 * f, 0] = 2.0 ** f
        At[2 * f + 1, 0] = 2.0 ** f
        At[2 * f + 1, 1] = np.pi / 2.0
    c['AtT'] = At.T.astype(f32).copy()                           # [2,12]

    perm63 = list(range(3, 63)) + [0, 1, 2]
    c['pW0sin'] = np.ascontiguousarray(inp['pW0'][3:63])         # [60,128]
    c['pW0lin'] = np.ascontiguousarray(inp['pW0'][0:3])          # [3,128]
    c['pW1'] = inp['pW1'].copy()
    c['pW2'] = inp['pW2'].copy()
    c['pWo'] = inp['pWo'].copy()                                 # [128,1]
    c['pb0col'] = inp['pb0'].reshape(-1, 1).copy()
    c['pb1col'] = inp['pb1'].reshape(-1, 1).copy()
    c['pb2col'] = inp['pb2'].reshape(-1, 1).copy()

    c['fW0my'] = np.ascontiguousarray(inp['fW0'][perm63])        # [63,256]

    def pack_km(Wm):  # [256, 256] -> [128, 4, 128], slot 2k+m
        out = np.zeros((128, 4, 128), f32)
        for k in range(2):
            for m in range(2):
                out[:, 2 * k + m, :] = Wm[k * 128:(k + 1) * 128,
                                          m * 128:(m + 1) * 128]
        return out

    for i in range(3):
        c[f'fWm{i}'] = pack_km(inp['fWm'][i])
        c[f'fWp{i}'] = pack_km(inp['fWp'][i])
    c['fWs_h'] = pack_km(inp['fWs'][0:256])
    c['fWs_e'] = np.ascontiguousarray(inp['fWs'][256:][perm63])  # [63,256]
    c['fb0col'] = inp['fb0'].reshape(2, 128).T.copy()            # [128,2]
    for i in range(3):
        c[f'fbm{i}col'] = inp['fbm'][i].reshape(2, 128).T.copy()
        c[f'fbp{i}col'] = inp['fbp'][i].reshape(2, 128).T.copy()
    c['fbscol'] = inp['fbs'].reshape(2, 128).T.copy()

    # view head: fold Wfeat into Wview
    Wv = inp['Wview']
    Wv_d, Wv_emb, Wv_t, Wv_app = (Wv[256:283], Wv[283:331],
                                  Wv[331:344], Wv[344:383])
    Wfc = (inp['Wfeat'].astype(np.float64) @ Wv[0:256].astype(np.float64)
           ).astype(f32)
    out = np.zeros((128, 2, 128), f32)
    out[:, 0, :] = Wfc[0:128]
    out[:, 1, :] = Wfc[128:256]
    c['Wfc'] = out
    c['bveffcol'] = (inp['bfeat'].astype(np.float64)
                     @ Wv[0:256].astype(np.float64)
                     + inp['bview'].astype(np.float64)
                     ).astype(f32).reshape(-1, 1)
    perm39 = list(range(3, 39)) + [0, 1, 2]
    c['Wv_app'] = np.ascontiguousarray(Wv_app[perm39])           # [39,128]
    c['Wv_d_lin'] = np.ascontiguousarray(Wv_d[0:3])
    c['Wv_d_sin'] = np.ascontiguousarray(Wv_d[3:27])
    c['Wv_emb'] = np.ascontiguousarray(Wv_emb)
    c['Wv_t_lin'] = np.ascontiguousarray(Wv_t[0:1])
    c['Wv_t_sin'] = np.ascontiguousarray(Wv_t[1:13])
    c['Wsig'] = np.stack([inp['Wsig'][0:128, 0],
                          inp['Wsig'][128:256, 0]], 1).copy()    # [128,2]
    c['Wrgb'] = inp['Wrgb'].copy()                               # [128,3]
    c['brgbcol'] = inp['brgb'].reshape(-1, 1).copy()             # [3,1]
    c['emb_table'] = inp['emb_table'].copy()

    c['sgrid'] = np.broadcast_to(
        np.arange(129, dtype=f32) / 128.0, (128, 129)).copy()
    c['identity'] = np.eye(128, dtype=f32)
    E = np.zeros((4, 512), f32)
    for rl in range(4):
        E[rl, rl * 128:(rl + 1) * 128] = 1.0
    c['Etile'] = E
    c['iotacol'] = np.arange(100, dtype=f32).reshape(-1, 1)
    scalars = dict(pbo_f=float(inp['pbo'][0]), bsig_f=float(inp['bsig'][0]))
    return c, scalars


INPUT_SHAPES = {
    'rays': (R, 12),
    'cA3selT': (3, 60), 'cphasecol': (60, 1),
    'fA3T': (3, 106), 'fA4T': (4, 106),
    'AdT': (4, 24), 'AtT': (2, 12),
    'pW0sin': (60, 128), 'pW0lin': (3, 128),
    'pW1': (128, 128), 'pW2': (128, 128), 'pWo': (128, 1),
    'pb0col': (128, 1), 'pb1col': (128, 1), 'pb2col': (128, 1),
    'fW0my': (63, 256), 'fWm0': (128, 4, 128), 'fWm1': (128, 4, 128),
    'fWm2': (128, 4, 128), 'fWp0': (128, 4, 128), 'fWp1': (128, 4, 128),
    'fWp2': (128, 4, 128), 'fWs_h': (128, 4, 128), 'fWs_e': (63, 256),
    'fb0col': (128, 2), 'fbm0col': (128, 2), 'fbm1col': (128, 2),
    'fbm2col': (128, 2), 'fbp0col': (128, 2), 'fbp1col': (128, 2),
    'fbp2col': (128, 2), 'fbscol': (128, 2),
    'Wfc': (128, 2, 128), 'bveffcol': (128, 1), 'Wv_app': (39, 128),
    'Wv_d_lin': (3, 128), 'Wv_d_sin': (24, 128), 'Wv_emb': (48, 128),
    'Wv_t_lin': (1, 128), 'Wv_t_sin': (12, 128),
    'Wsig': (128, 2), 'Wrgb': (128, 3), 'brgbcol': (3, 1),
    'emb_table': (100, 48),
    'sgrid': (128, 129), 'identity': (128, 128),
    'Etile': (4, 512), 'iotacol': (100, 1),
}
F32R_WEIGHTS = {'fW0my', 'fWm0', 'fWm1', 'fWm2', 'fWp0', 'fWp1', 'fWp2',
                'fWs_h', 'fWs_e', 'Wfc', 'Wv_app', 'Wv_d_lin', 'Wv_d_sin',
                'Wv_emb', 'Wv_t_lin', 'Wv_t_sin', 'Wsig', 'Wrgb',
                'emb_table', 'Etile'}


# ---------------------------------------------------------------- bass build
def build_nc(pbo_f, bsig_f, stage=3, debug=False):
    nc = bacc.Bacc("TRN2", target_bir_lowering=False)
    D = {k: nc.dram_tensor(k, list(v), F32, kind="ExternalInput")
         for k, v in INPUT_SHAPES.items()}
    OUT = nc.dram_tensor("rgb_out", [R, 3], F32, kind="ExternalOutput")
    dbg = {}
    if debug:
        for nm, shp in [("d_sigc", (R, S)), ("d_zf", (R, S + 1)),
                        ("d_wc", (R, S)), ("d_sigf", (R, S)),
                        ("d_wf", (R, S)), ("d_hvray", (128, R)),
                        ("d_ec", (63, CN)), ("d_efa", (63, CN)),
                        ("d_efb", (39, CN)), ("d_h1", (128, 2 * TILE_N))]:
            dbg[nm] = nc.dram_tensor(nm, list(shp), F32, kind="ExternalOutput")
    with tile.TileContext(nc) as tc:
        _body(nc, tc, D, OUT, dbg, pbo_f, bsig_f, stage, debug)
    nc.compile()
    return nc


def _body(nc, tc, D, OUT, dbg, pbo_f, bsig_f, stage, debug):
    from contextlib import ExitStack
    ctx = ExitStack()
    wpool = ctx.enter_context(tc.tile_pool(name="w", bufs=1))
    per = ctx.enter_context(tc.tile_pool(name="per", bufs=1))
    pp2 = ctx.enter_context(tc.tile_pool(name="pp2", bufs=2))
    big = ctx.enter_context(tc.tile_pool(name="big", bufs=2))
    hp = ctx.enter_context(tc.tile_pool(name="h", bufs=3))
    dram = ctx.enter_context(tc.tile_pool(name="dr", bufs=2, space="DRAM"))
    psA = ctx.enter_context(tc.tile_pool(name="psA", bufs=3, space="PSUM"))
    psS = ctx.enter_context(tc.tile_pool(name="psS", bufs=2, space="PSUM"))
    psR = ctx.enter_context(tc.tile_pool(name="psR", bufs=1, space="PSUM"))
    psC = ctx.enter_context(tc.tile_pool(name="psC", bufs=1, space="PSUM"))

    W = {}
    for k, t in D.items():
        if k == 'rays':
            continue
        dt = F32R if k in F32R_WEIGHTS else F32
        tl = wpool.tile(list(t.shape), dt, tag="w_" + k)
        nc.sync.dma_start(tl[:], t[:].bitcast(F32R) if dt == F32R else t[:])
        W[k] = tl
    rays = wpool.tile([R, 12], F32, tag="w_rays")
    nc.sync.dma_start(rays[:], D['rays'][:])
    ident = W['identity']

    # ---------------- phase 0: per-ray prep (ray-major layout)
    nearc = per.tile([R, 1], F32)
    nc.vector.tensor_scalar(nearc[:], rays[:, 6:7], 1e-8, None, op0=OP.max)
    spanc = per.tile([R, 1], F32)
    nc.vector.tensor_tensor(spanc[:], rays[:, 7:8], nearc[:], op=OP.subtract)

    dsq = per.tile([R, 3], F32)
    nc.vector.tensor_tensor(dsq[:], rays[:, 3:6], rays[:, 3:6], op=OP.mult)
    ssum = per.tile([R, 1], F32)
    nc.vector.reduce_sum(ssum[:], dsq[:], axis=mybir.AxisListType.X)
    norm = per.tile([R, 1], F32)
    nc.scalar.activation(norm[:], ssum[:], AF.Sqrt)
    for it in range(2):
        t1 = per.tile([R, 1], F32, tag="nwt")
        nc.vector.reciprocal(t1[:], norm[:])
        nc.vector.scalar_tensor_tensor(t1[:], ssum[:], 1.0, t1[:],
                                       op0=OP.mult, op1=OP.mult)
        nc.vector.tensor_tensor(t1[:], t1[:], norm[:], op=OP.add)
        nc.vector.tensor_scalar(norm[:], t1[:], 0.5, None, op0=OP.mult)
    invn = per.tile([R, 1], F32)
    nc.vector.reciprocal(invn[:], norm[:])

    # bundle: 0:3 oc, 3 ones | 4:7 dc | 8:11 o, 11 ones | 12:15 dir |
    #         16:19 viewdir, 19 ones | 20 t, 21 ones | 22 embid
    bundle = per.tile([R, 28], F32)
    nc.gpsimd.memset(bundle[:], 0.0)
    nc.vector.scalar_tensor_tensor(bundle[:, 0:3], rays[:, 3:6], nearc[:],
                                   rays[:, 0:3], op0=OP.mult, op1=OP.add)
    nc.vector.memset(bundle[:, 3:4], 1.0)
    nc.vector.tensor_scalar(bundle[:, 4:7], rays[:, 3:6], spanc[:], None,
                            op0=OP.mult)
    nc.vector.tensor_copy(bundle[:, 8:11], rays[:, 0:3])
    nc.vector.memset(bundle[:, 11:12], 1.0)
    nc.vector.tensor_copy(bundle[:, 12:15], rays[:, 3:6])
    nc.vector.tensor_scalar(bundle[:, 16:19], rays[:, 3:6], invn[:], None,
                            op0=OP.mult)
    nc.vector.memset(bundle[:, 19:20], 1.0)
    nc.vector.tensor_copy(bundle[:, 20:21], rays[:, 8:9])
    nc.vector.memset(bundle[:, 21:22], 1.0)
    nc.vector.tensor_copy(bundle[:, 22:23], rays[:, 9:10])

    def transp(col):
        p = psC.tile([4, 128], F32, tag="ptp")
        nc.tensor.transpose(p[:], bundle[:, col:col + 4], ident[:])
        sb = per.tile([4, 128], F32, tag="tp%d" % col)
        nc.scalar.copy(sb[:], p[:])
        return sb

    ocT = transp(0)      # [ocT;ones]
    dcT = transp(4)      # [dcT;..]
    oT = transp(8)       # [oT;ones]
    dirT = transp(12)
    vdT = transp(16)     # [viewdirT;ones]
    tT = transp(20)      # [t;ones;embid]
    eiT = transp(22)     # row0 = embid (base 0 for partition_broadcast)

    def mm_copy(lhsT, rhs, shape, nm, dst_dtype=F32):
        p = psC.tile(shape, F32, tag="pmc")
        nc.tensor.matmul(p[:], lhsT, rhs, start=True, stop=True)
        sb = per.tile(shape, dst_dtype, tag="mc_" + nm)
        nc.scalar.copy(sb[:], p[:])
        return sb

    Bf = mm_copy(W['fA3T'][:], dirT[0:3, :], [106, 128], "Bf")
    Cf = mm_copy(W['fA4T'][:], oT[:], [106, 128], "Cf")

    def rangered_v(ap, shape, tag):
        sc = per.tile(shape, F32, tag=tag)
        nc.vector.tensor_scalar(sc[:], ap, float(INV2PI), float(MAGIC),
                                op0=OP.mult, op1=OP.add)
        nc.vector.tensor_scalar(sc[:], sc[:], float(MAGIC), None,
                                op0=OP.subtract)
        nc.vector.scalar_tensor_tensor(ap, sc[:], -float(C1), ap,
                                       op0=OP.mult, op1=OP.add)
        nc.vector.scalar_tensor_tensor(ap, sc[:], -float(C2), ap,
                                       op0=OP.mult, op1=OP.add)

    # per-ray view features
    argd = mm_copy(W['AdT'][:], vdT[:], [24, 128], 'argd')
    rangered_v(argd[:], [24, 128], "rrd")
    sind = per.tile([24, 128], F32R)
    nc.scalar.activation(sind[:], argd[:], AF.Sin)
    vd_r = per.tile([4, 128], F32R)
    nc.vector.tensor_copy(vd_r[:], vdT[:])

    argt = mm_copy(W['AtT'][:], tT[0:2, :], [12, 128], 'argt')
    rangered_v(argt[:], [12, 128], "rrt")
    sint = per.tile([12, 128], F32R)
    nc.scalar.activation(sint[:], argt[:], AF.Sin)
    t_r = per.tile([4, 128], F32R)
    nc.vector.tensor_copy(t_r[:], tT[:])

    embBC = per.tile([100, 128], F32)
    nc.gpsimd.partition_broadcast(embBC[:], eiT[0:1, :], channels=100)
    onehot = per.tile([100, 128], F32R)
    nc.vector.tensor_scalar(onehot[:], embBC[:], W['iotacol'][:], None,
                            op0=OP.is_equal)
    embT = mm_copy(W['emb_table'][:], onehot[:], [48, 128], 'embT', dst_dtype=F32R)

    phv = psC.tile([128, 128], F32, tag="pmc")
    nc.tensor.matmul(phv[:], W['Wv_d_lin'][:], vd_r[0:3, :],
                     start=True, stop=False)
    nc.tensor.matmul(phv[:], W['Wv_d_sin'][:], sind[:], start=False, stop=False)
    nc.tensor.matmul(phv[:], W['Wv_emb'][:], embT[:], start=False, stop=False)
    nc.tensor.matmul(phv[:], W['Wv_t_lin'][:], t_r[0:1, :],
                     start=False, stop=False)
    nc.tensor.matmul(phv[:], W['Wv_t_sin'][:], sint[:], start=False, stop=True)
    hvray = per.tile([128, 128], F32)
    nc.vector.tensor_scalar(hvray[:], phv[:], W['bveffcol'][:], None,
                            op0=OP.add)
    if debug:
        nc.sync.dma_start(dbg["d_hvray"][:], hvray[:])
    phvT = psC.tile([128, 128], F32, tag="pmc")
    nc.tensor.transpose(phvT[:], hvray[:], ident[:])
    hvrayT = per.tile([128, 128], F32R)
    nc.scalar.copy(hvrayT[:], phvT[:])
    hvb = dram.tile([128, 128], F32R, tag="hvb")
    nc.sync.dma_start(hvb[:], hvrayT[:])
    hvre = wpool.tile([4, 32, 128], F32R, tag="hvre")
    nc.sync.dma_start(hvre[:], hvb[:].rearrange("(t rl) m -> rl t m", rl=4))

    # coarse z edges
    zc = per.tile([R, S + 1], F32)
    nc.vector.tensor_scalar(zc[:], W['sgrid'][:], spanc[:], None, op0=OP.mult)
    nc.vector.tensor_scalar(zc[:], zc[:], nearc[:], None, op0=OP.add)
    midc = per.tile([R, S], F32)
    nc.vector.tensor_tensor(midc[:], zc[:, 0:S], zc[:, 1:S + 1], op=OP.add)
    nc.vector.tensor_scalar(midc[:], midc[:], 0.5, None, op0=OP.mult)

    # ======================= COARSE PASS =======================
    sigcT = per.tile([R, S], F32, tag="sigcT")
    for ci in range(NCHUNK):
        r0 = ci * CHUNK_RAYS
        mbc = dram.tile([CHUNK_RAYS, S], F32, tag="midb")
        nc.sync.dma_start(mbc[:], midc[r0:r0 + CHUNK_RAYS, :])
        mfc = pp2.tile([1, CN], F32, tag="flat", bufs=1)
        nc.sync.dma_start(mfc[:],
                          mbc[:].rearrange("p f -> (p f)").unsqueeze(0))
        mx3 = pp2.tile([3, CN], F32, tag="mx3", bufs=1)
        nc.gpsimd.partition_broadcast(mx3[:], mfc[:], channels=3)
        xyzc = pp2.tile([3, CN], F32, tag="xyzc", bufs=1)
        d3 = dirT[0:3, r0:r0 + CHUNK_RAYS].unsqueeze(2).broadcast_to(
            [3, CHUNK_RAYS, S])
        o3 = oT[0:3, r0:r0 + CHUNK_RAYS].unsqueeze(2).broadcast_to(
            [3, CHUNK_RAYS, S])
        x3 = xyzc[:].rearrange("p (r s) -> p r s", r=CHUNK_RAYS)
        nc.vector.tensor_tensor(
            x3, mx3[:].rearrange("p (r s) -> p r s", r=CHUNK_RAYS),
            d3, op=OP.mult)
        nc.vector.tensor_tensor(x3, x3, o3, op=OP.add)

        ec = big.tile([60, CN], F32, tag="arg")
        for t in range(NTILE):
            colsq = slice(t * TILE_N, (t + 1) * TILE_N)
            pa = psA.tile([60, TILE_N], F32, tag="mmps", name="pa")
            nc.tensor.matmul(pa[:], W['cA3selT'][:], xyzc[:, colsq],
                             start=True, stop=True)
            nc.vector.tensor_scalar(ec[:, colsq], pa[:], W['cphasecol'][:],
                                    None, op0=OP.add)
        sc = big.tile([96, CN], F32, tag="mbcrr")
        nc.gpsimd.tensor_scalar(sc[0:60, :], ec[0:60, :], float(INV2PI),
                                float(MAGIC), op0=OP.mult, op1=OP.add)
        nc.gpsimd.tensor_scalar(sc[0:60, :], sc[0:60, :], float(MAGIC), None,
                                op0=OP.subtract)
        nc.vector.scalar_tensor_tensor(ec[0:60, :], sc[0:60, :], -float(C1),
                                       ec[0:60, :], op0=OP.mult, op1=OP.add)
        nc.vector.scalar_tensor_tensor(ec[0:60, :], sc[0:60, :], -float(C2),
                                       ec[0:60, :], op0=OP.mult, op1=OP.add)
        nc.scalar.activation(ec[0:60, :], ec[0:60, :], AF.Sin)

        sb_ = dram.tile([1, CN], F32, tag="sigb")
        sigflat = pp2.tile([1, CN], F32, tag="sigflat", bufs=1)
        for t in range(NTILE):
            cols = slice(t * TILE_N, (t + 1) * TILE_N)
            p1 = psA.tile([128, TILE_N], F32, tag="mmps")
            nc.tensor.matmul(p1[:], W['pW0sin'][:], ec[:, cols],
                             start=True, stop=False)
            nc.tensor.matmul(p1[:], W['pW0lin'][:], xyzc[:, cols],
                             start=False, stop=True)
            h1 = hp.tile([128, TILE_N], F32, tag="ch", bufs=2)
            nc.scalar.activation(h1[:], p1[:], AF.Relu, bias=W['pb0col'][:])
            p2 = psA.tile([128, TILE_N], F32, tag="mmps")
            nc.tensor.matmul(p2[:], W['pW1'][:], h1[:], start=True, stop=True)
            h2 = hp.tile([128, TILE_N], F32, tag="ch", bufs=2)
            nc.vector.tensor_scalar(h2[:], p2[:], W['pb1col'][:], 0.0,
                                    op0=OP.add, op1=OP.max)
            p3 = psA.tile([128, TILE_N], F32, tag="mmps")
            nc.tensor.matmul(p3[:], W['pW2'][:], h2[:], start=True, stop=True)
            h3 = hp.tile([128, TILE_N], F32, tag="ch", bufs=2)
            nc.scalar.activation(h3[:], p3[:], AF.Relu, bias=W['pb2col'][:])
            ps_ = psS.tile([1, TILE_N], F32, tag="sigps")
            nc.tensor.matmul(ps_[:], W['pWo'][:], h3[:], start=True, stop=True)
            if t % 2 == 0:
                nc.scalar.copy(sigflat[0:1, cols], ps_[:])
            else:
                nc.vector.tensor_copy(sigflat[0:1, cols], ps_[:])
        nc.sync.dma_start(sb_[:], sigflat[:])
        nc.sync.dma_start(sigcT[r0:r0 + CHUNK_RAYS, :],
                          sb_[:].rearrange("a (p f) -> (a p) f", p=CHUNK_RAYS))

    if debug:
        nc.sync.dma_start(dbg["d_sigc"][:], sigcT[:])
    if stage < 2:
        ctx.close()
        return

    # ======================= raw2weights helper =======================
    def raw2w(sigT_ap, z_lo, z_hi, norm_ap, bias_f, nrows, tag):
        """w = alpha * exclusive-cumprod(1-alpha+1e-10); returns (w, dz)."""
        P = nrows
        dz = per.tile([P, S], F32, tag=tag + "dz")
        nc.vector.tensor_tensor(dz[:], z_hi, z_lo, op=OP.subtract)
        di = per.tile([P, S], F32, tag=tag + "di")
        nc.vector.tensor_scalar(di[:], dz[:], norm_ap, None, op0=OP.mult)
        s1 = per.tile([P, S], F32, tag=tag + "s1")
        nc.vector.tensor_scalar(s1[:], sigT_ap, bias_f, 0.0,
                                op0=OP.add, op1=OP.max)
        ea = per.tile([P, S], F32, tag=tag + "ea")
        nc.vector.tensor_tensor(ea[:], s1[:], di[:], op=OP.mult)
        e = per.tile([P, S], F32, tag=tag + "e")
        nc.scalar.activation(e[:], ea[:], AF.Exp, scale=-1.0)
        al = per.tile([P, S], F32, tag=tag + "al")
        nc.vector.tensor_scalar(al[:], e[:], -1.0, 1.0, op0=OP.mult, op1=OP.add)
        om = per.tile([P, S], F32, tag=tag + "om")
        nc.vector.tensor_scalar(om[:], e[:], 1e-10, None, op0=OP.add)
        tr = per.tile([P, S], F32, tag=tag + "tr")
        nc.vector.tensor_tensor_scan(tr[:], om[:], om[:], 1.0,
                                     op0=OP.mult, op1=OP.bypass)
        w = per.tile([P, S], F32, tag=tag + "w")
        nc.vector.tensor_copy(w[:, 0:1], al[:, 0:1])
        nc.vector.tensor_tensor(w[:, 1:S], al[:, 1:S], tr[:, 0:S - 1],
                                op=OP.mult)
        return w, dz

    wc, dzc = raw2w(sigcT[:], zc[:, 0:S], zc[:, 1:S + 1], norm[:],
                    pbo_f, R, "c")
    if debug:
        nc.sync.dma_start(dbg["d_wc"][:], wc[:])

    # ======================= sample_pdf =======================
    Wt = per.tile([R, S], F32)
    nc.vector.tensor_scalar(Wt[:], wc[:], 1e-5, None, op0=OP.add)
    Sx = per.tile([R, S], F32)
    nc.vector.memset(Sx[:, 0:1], 0.0)
    nc.vector.tensor_tensor_scan(Sx[:, 1:S], Wt[:, 0:S - 1], Wt[:, 0:S - 1],
                                 0.0, op0=OP.add, op1=OP.bypass)
    Tt = per.tile([R, 1], F32)
    nc.vector.tensor_tensor(Tt[:], Sx[:, S - 1:S], Wt[:, S - 1:S], op=OP.add)
    G2 = per.tile([R, S], F32)
    nc.vector.reciprocal(G2[:], Wt[:])
    Sn = per.tile([R, S], F32)
    nc.vector.tensor_scalar(Sn[:], Sx[:], -1.0, None, op0=OP.mult)
    UT = per.tile([R, S + 1], F32)
    nc.vector.tensor_scalar(UT[:], W['sgrid'][:], Tt[:], None, op0=OP.mult)
    zf = per.tile([R, S + 1], F32)
    for j in range(S + 1):
        x_ = pp2.tile([R, S], F32, tag="pdfx")
        nc.vector.scalar_tensor_tensor(x_[:], Sn[:], UT[:, j:j + 1], G2[:],
                                       op0=OP.add, op1=OP.mult)
        xc_ = pp2.tile([R, S], F32, tag="pdfxc")
        nc.vector.tensor_scalar(xc_[:], x_[:], 0.0, 1.0,
                                op0=OP.max, op1=OP.min)
        sc_ = pp2.tile([R, S], F32, tag="pdfsc")
        nc.vector.scalar_tensor_tensor(sc_[:], xc_[:], 1.0, dzc[:],
                                       op0=OP.mult, op1=OP.mult,
                                       accum_out=zf[:, j:j + 1])
    nc.vector.tensor_scalar(zf[:], zf[:], zc[:, 0:1], None, op0=OP.add)
    if debug:
        nc.sync.dma_start(dbg["d_zf"][:], zf[:])
    if stage < 3:
        ctx.close()
        return

    midf = per.tile([R, S], F32)
    nc.vector.tensor_tensor(midf[:], zf[:, 0:S], zf[:, 1:S + 1], op=OP.add)
    nc.vector.tensor_scalar(midf[:], midf[:], 0.5, None, op0=OP.mult)

    # ======================= FINE PASS =======================
    rgbmT = per.tile([3, 128], F32)
    nc.vector.memset(rgbmT[:], 0.0)

    for ci in range(NCHUNK):
        r0 = ci * CHUNK_RAYS
        mb = dram.tile([CHUNK_RAYS, S], F32, tag="midb")
        nc.sync.dma_start(mb[:], midf[r0:r0 + CHUNK_RAYS, :])
        mflat = pp2.tile([1, CN], F32, tag="flat", bufs=1)
        nc.sync.dma_start(mflat[:],
                          mb[:].rearrange("p f -> (p f)").unsqueeze(0))
        mBC = big.tile([106, CN], F32, tag="mbcrr")
        nc.gpsimd.partition_broadcast(mBC[:], mflat[:], channels=106)

        argf = big.tile([106, CN], F32, tag="arg")
        b3 = Bf[:, r0:r0 + CHUNK_RAYS].unsqueeze(2).broadcast_to(
            [106, CHUNK_RAYS, S])
        c3 = Cf[:, r0:r0 + CHUNK_RAYS].unsqueeze(2).broadcast_to(
            [106, CHUNK_RAYS, S])
        a3 = argf[:].rearrange("p (r s) -> p r s", r=CHUNK_RAYS)
        m3 = mBC[:].rearrange("p (r s) -> p r s", r=CHUNK_RAYS)
        nc.vector.tensor_tensor(a3, m3, b3, op=OP.mult)
        nc.gpsimd.tensor_tensor(a3, a3, c3, op=OP.add)
        sc = big.tile([100, CN], F32, tag="mbcrr")
        for lo, hi in ((0, 60), (64, 100)):
            nc.gpsimd.tensor_scalar(sc[lo:hi, :], argf[lo:hi, :], float(INV2PI),
                                    float(MAGIC), op0=OP.mult, op1=OP.add)
            nc.gpsimd.tensor_scalar(sc[lo:hi, :], sc[lo:hi, :], float(MAGIC),
                                    None, op0=OP.subtract)
            nc.vector.scalar_tensor_tensor(argf[lo:hi, :], sc[lo:hi, :],
                                           -float(C1), argf[lo:hi, :],
                                           op0=OP.mult, op1=OP.add)
            nc.vector.scalar_tensor_tensor(argf[lo:hi, :], sc[lo:hi, :],
                                           -float(C2), argf[lo:hi, :],
                                           op0=OP.mult, op1=OP.add)
        efa = big.tile([63, CN], F32R, tag="efa")
        efb = big.tile([39, CN], F32R, tag="efb")
        nc.scalar.activation(efa[0:60, :], argf[0:60, :], AF.Sin)
        nc.scalar.activation(efb[0:36, :], argf[64:100, :], AF.Sin)
        nc.sync.dma_start(efa[60:63, :], argf[100:103, :].bitcast(F32R))
        nc.sync.dma_start(efb[36:39, :], argf[103:106, :].bitcast(F32R))
        if debug and ci == 0:
            nc.sync.dma_start(dbg["d_efa"][:], efa[:].bitcast(F32))
            nc.sync.dma_start(dbg["d_efb"][:], efb[:].bitcast(F32))

        rgbS = big.tile([3, CN], F32, tag="rgbS")
        sb_ = dram.tile([1, CN], F32, tag="sigb")
        sigflat = pp2.tile([1, CN], F32, tag="sigflat", bufs=1)
        for t in range(NTILE):
            cols = slice(t * TILE_N, (t + 1) * TILE_N)
            gtile = ci * NTILE + t

            pm = [psA.tile([128, TILE_N], F32, tag="mmps", name="pm%d" % _m)
                  for _m in range(2)]
            for m in range(2):
                nc.tensor.matmul(pm[m][:], W['fW0my'][:, m * 128:(m + 1) * 128],
                                 efa[:, cols], start=True, stop=True)
            h = hp.tile([128, 2 * TILE_N], F32R, tag="fh")
            nc.scalar.activation(h[:, 0:TILE_N], pm[0][:], AF.Relu,
                                 bias=W['fb0col'][:, 0:1])
            nc.vector.tensor_scalar(h[:, TILE_N:], pm[1][:],
                                    W['fb0col'][:, 1:2], 0.0,
                                    op0=OP.add, op1=OP.max)
            if debug and ci == 0 and t == 0:
                nc.sync.dma_start(dbg["d_h1"][:], h[:].bitcast(F32))

            def mid_layer(wname, bname, hin, skip=False):
                pmm = [psA.tile([128, TILE_N], F32, tag="mmps",
                                name="pmm%d" % _m) for _m in range(2)]
                for m in range(2):
                    nc.tensor.matmul(pmm[m][:], W[wname][:, m, :],
                                     hin[:, 0:TILE_N], start=True, stop=False)
                    nc.tensor.matmul(pmm[m][:], W[wname][:, 2 + m, :],
                                     hin[:, TILE_N:],
                                     start=False, stop=not skip)
                    if skip:
                        nc.tensor.matmul(pmm[m][:],
                                         W['fWs_e'][:, m * 128:(m + 1) * 128],
                                         efa[:, cols], start=False, stop=True)
                hout = hp.tile([128, 2 * TILE_N], F32R, tag="fh")
                nc.scalar.activation(hout[:, 0:TILE_N], pmm[0][:], AF.Relu,
                                     bias=W[bname][:, 0:1])
                nc.vector.tensor_scalar(hout[:, TILE_N:], pmm[1][:],
                                        W[bname][:, 1:2], 0.0,
                                        op0=OP.add, op1=OP.max)
                return hout

            h = mid_layer('fWm0', 'fbm0col', h)
            h = mid_layer('fWm1', 'fbm1col', h)
            h = mid_layer('fWm2', 'fbm2col', h)
            h = mid_layer('fWs_h', 'fbscol', h, skip=True)
            h = mid_layer('fWp0', 'fbp0col', h)
            h = mid_layer('fWp1', 'fbp1col', h)
            h = mid_layer('fWp2', 'fbp2col', h)

            ps_ = psS.tile([1, TILE_N], F32, tag="sigps")
            nc.tensor.matmul(ps_[:], W['Wsig'][:, 0:1], h[:, 0:TILE_N],
                             start=True, stop=False)
            nc.tensor.matmul(ps_[:], W['Wsig'][:, 1:2], h[:, TILE_N:],
                             start=False, stop=True)
            if t % 2 == 0:
                nc.scalar.copy(sigflat[0:1, cols], ps_[:])
            else:
                nc.vector.tensor_copy(sigflat[0:1, cols], ps_[:])

            pv = psA.tile([128, TILE_N], F32, tag="mmps")
            nc.tensor.matmul(pv[:], W['Wfc'][:, 0, :], h[:, 0:TILE_N],
                             start=True, stop=False)
            nc.tensor.matmul(pv[:], W['Wfc'][:, 1, :], h[:, TILE_N:],
                             start=False, stop=False)
            nc.tensor.matmul(pv[:], W['Wv_app'][:], efb[:, cols],
                             start=False, stop=False)
            nc.tensor.matmul(pv[:], hvre[:, gtile, :], W['Etile'][:],
                             start=False, stop=True)
            hv = hp.tile([128, TILE_N], F32R, tag="fhv", bufs=2)
            nc.vector.tensor_scalar(hv[:], pv[:], 0.0, None, op0=OP.max)

            prgb = psR.tile([3, TILE_N], F32, tag="rgbps")
            nc.tensor.matmul(prgb[:], W['Wrgb'][:], hv[:],
                             start=True, stop=True)
            nc.scalar.activation(rgbS[0:3, cols], prgb[:],
                                 AF.Sigmoid, bias=W['brgbcol'][:])

        nc.sync.dma_start(sb_[:], sigflat[:])
        sigch = pp2.tile([CHUNK_RAYS, S], F32, tag="sigch")
        nc.sync.dma_start(sigch[:],
                          sb_[:].rearrange("a (p f) -> (a p) f", p=CHUNK_RAYS))
        zfc = pp2.tile([CHUNK_RAYS, S + 1], F32, tag="zfc")
        nc.sync.dma_start(zfc[:], zf[r0:r0 + CHUNK_RAYS, :])
        normc = pp2.tile([CHUNK_RAYS, 1], F32, tag="normc")
        nc.sync.dma_start(normc[:], norm[r0:r0 + CHUNK_RAYS, :])

        wf, _dzf = raw2w(sigch[:], zfc[:, 0:S], zfc[:, 1:S + 1],
                         normc[:], bsig_f, CHUNK_RAYS, "f")
        if debug:
            nc.sync.dma_start(dbg["d_sigf"][r0:r0 + CHUNK_RAYS, :], sigch[:])
            nc.sync.dma_start(dbg["d_wf"][r0:r0 + CHUNK_RAYS, :], wf[:])

        wb = dram.tile([CHUNK_RAYS, S], F32, tag="wb")
        nc.sync.dma_start(wb[:], wf[:])
        wflat = pp2.tile([1, CN], F32, tag="flat", bufs=1)
        nc.sync.dma_start(wflat[:],
                          wb[:].rearrange("p f -> (p f)").unsqueeze(0))
        wBC = big.tile([3, CN], F32, tag="arg")
        nc.gpsimd.partition_broadcast(wBC[:], wflat[:], channels=3)
        nc.vector.tensor_tensor(rgbS[0:3, :], rgbS[0:3, :], wBC[0:3, :],
                                op=OP.mult)
        nc.vector.tensor_reduce(
            rgbmT[0:3, r0:r0 + CHUNK_RAYS],
            rgbS[0:3, :].rearrange("p (r s) -> p r s", r=CHUNK_RAYS),
            axis=mybir.AxisListType.X, op=OP.add)

    # out: transpose [3,128] -> [128,3] via DRAM bounce
    rb = dram.tile([3, 128], F32, tag="rb")
    nc.sync.dma_start(rb[:], rgbmT[:])
    rgbout = per.tile([128, 3], F32)
    nc.sync.dma_start(rgbout[:], rb[:].rearrange("c r -> r c"))
    nc.sync.dma_start(OUT[:], rgbout[:])
    ctx.close()


# ---------------------------------------------------------------- entry
_CACHE = {}


def kernel(**inputs):
    inp = {k: np.asarray(v) for k, v in inputs.items()}
    consts, scal = host_prep(inp)
    key = (BUILD_STAGE, DEBUG_OUT, scal['pbo_f'], scal['bsig_f'])
    if key not in _CACHE:
        _CACHE[key] = build_nc(scal['pbo_f'], scal['bsig_f'],
                               stage=BUILD_STAGE, debug=DEBUG_OUT)
    nc = _CACHE[key]
    rays = np.asarray(inp['rays'], np.float32)
    in_maps = []
    for core in range(NCORES):
        m = {k: np.ascontiguousarray(v, dtype=np.float32)
             for k, v in consts.items()}
        m['rays'] = np.ascontiguousarray(rays[core * R:(core + 1) * R])
        in_maps.append(m)
    res = run_bass_kernel_spmd(nc, in_maps, core_ids=list(range(NCORES)))
    globals()['_LAST_RESULTS'] = res
    return np.concatenate([r['rgb_out'] for r in res.results], 0)


# revision 18
# speedup vs baseline: 1.2643x; 1.2643x over previous
"""NeRF-style render kernel for TRN2 (8 NeuronCores, data-parallel over rays).

Self-contained: hardcodes all shapes. Coarse proposal MLP runs in fp32
(resampling is precision-critical), fine MLP in float32r.
"""
import os
import sys

sys.path.insert(0, '/opt/trn_rl_repo')
import numpy as np
import concourse.bass as bass
import concourse.bacc as bacc
import concourse.tile as tile
import concourse.mybir as mybir
from concourse.bass_utils import run_bass_kernel_spmd

F32 = mybir.dt.float32
F32R = mybir.dt.float32r
AF = mybir.ActivationFunctionType
OP = mybir.AluOpType

NCORES = 8
R = 128          # rays per core
S = 128          # samples per pass
CHUNK_RAYS = 16  # rays per chunk
NCHUNK = R // CHUNK_RAYS          # 8
CN = CHUNK_RAYS * S               # 2048 cols per chunk
TILE_N = 512                      # matmul moving size
NTILE = CN // TILE_N              # 4 point-tiles per chunk

MAGIC = np.float32(12582912.0)    # 1.5 * 2^23 (round-to-int trick)
INV2PI = np.float32(1.0 / (2.0 * np.pi))
C1 = np.float32(6.28125)          # 2*pi split, k*C1 exact for k < 2^13
C2 = np.float32(2.0 * np.pi - 6.28125)

BUILD_STAGE = int(os.environ.get("KERNEL_STAGE", "3"))
DEBUG_OUT = os.environ.get("KERNEL_DEBUG", "0") == "1"


# ---------------------------------------------------------------- host prep
def _posenc_rows(nf, span=None, minp=None):
    """A3 [6*nf,3] / const [6*nf] for rows f-major: per f: 3 sin, 3 cos."""
    rows = 6 * nf
    A3 = np.zeros((rows, 3), np.float64)
    ph = np.zeros((rows,), np.float64)
    for f in range(nf):
        for k in range(6):
            r = 6 * f + k
            d = k % 3
            sc = 2.0 ** f
            if span is not None:
                A3[r, d] = sc / span[d]
                ph[r] = -sc * minp[d] / span[d]
            else:
                A3[r, d] = sc
            if k >= 3:
                ph[r] += np.pi / 2.0
    return A3, ph


def host_prep(inp):
    c = {}
    f32 = np.float32

    # coarse posenc: selector*2^f matrix [3,60] + phase col [60,1]
    A3s, phs = _posenc_rows(10)
    c['cA3selT'] = A3s.T.astype(f32).copy()                      # [3,60]
    c['cphasecol'] = np.asarray(phs, np.float64).astype(f32).reshape(-1, 1)

    # fine posenc rows: [sinx60, sinapp36, xyz3, appx3]
    minp = inp['min_point'].astype(np.float64)
    span = (inp['max_point'] - inp['min_point']).astype(np.float64)
    A3a, pha = _posenc_rows(6, span=span, minp=minp)
    pad4 = np.zeros((4, 3))
    fA3 = np.concatenate([A3s, pad4, A3a, np.eye(3), np.diag(1.0 / span)], 0)
    fph = np.concatenate([phs, np.zeros(4), pha, np.zeros(3), -minp / span], 0)
    c['fA3T'] = fA3.T.astype(f32).copy()                         # [3,106]
    c['fA4T'] = np.concatenate([fA3, fph[:, None]], 1).T.astype(f32).copy()

    # per-ray enc matrices (lhsT)
    Ad = np.zeros((24, 4), np.float64)
    for f in range(4):
        for k in range(6):
            r = 6 * f + k
            Ad[r, k % 3] = 2.0 ** f
            if k >= 3:
                Ad[r, 3] = np.pi / 2.0
    c['AdT'] = Ad.T.astype(f32).copy()                           # [4,24]
    At = np.zeros((12, 2), np.float64)
    for f in range(6):
        At[2 * f, 0] = 2.0 ** f
        At[2 * f + 1, 0] = 2.0 ** f
        At[2 * f + 1, 1] = np.pi / 2.0
    c['AtT'] = At.T.astype(f32).copy()                           # [2,12]

    perm63 = list(range(3, 63)) + [0, 1, 2]
    c['pW0sin'] = np.ascontiguousarray(inp['pW0'][3:63])         # [60,128]
    c['pW0lin'] = np.ascontiguousarray(inp['pW0'][0:3])          # [3,128]
    c['pW1'] = inp['pW1'].copy()
    c['pW2'] = inp['pW2'].copy()
    c['pWo'] = inp['pWo'].copy()                                 # [128,1]
    c['pb0col'] = inp['pb0'].reshape(-1, 1).copy()
    c['pb1col'] = inp['pb1'].reshape(-1, 1).copy()
    c['pb2col'] = inp['pb2'].reshape(-1, 1).copy()

    c['fW0my'] = np.ascontiguousarray(inp['fW0'][perm63])        # [63,256]

    def pack_km(Wm):  # [256, 256] -> [128, 4, 128], slot 2k+m
        out = np.zeros((128, 4, 128), f32)
        for k in range(2):
            for m in range(2):
                out[:, 2 * k + m, :] = Wm[k * 128:(k + 1) * 128,
                                          m * 128:(m + 1) * 128]
        return out

    for i in range(3):
        c[f'fWm{i}'] = pack_km(inp['fWm'][i])
        c[f'fWp{i}'] = pack_km(inp['fWp'][i])
    c['fWs_h'] = pack_km(inp['fWs'][0:256])
    c['fWs_e'] = np.ascontiguousarray(inp['fWs'][256:][perm63])  # [63,256]
    c['fb0col'] = inp['fb0'].reshape(2, 128).T.copy()            # [128,2]
    for i in range(3):
        c[f'fbm{i}col'] = inp['fbm'][i].reshape(2, 128).T.copy()
        c[f'fbp{i}col'] = inp['fbp'][i].reshape(2, 128).T.copy()
    c['fbscol'] = inp['fbs'].reshape(2, 128).T.copy()

    # view head: fold Wfeat into Wview
    Wv = inp['Wview']
    Wv_d, Wv_emb, Wv_t, Wv_app = (Wv[256:283], Wv[283:331],
                                  Wv[331:344], Wv[344:383])
    Wfc = (inp['Wfeat'].astype(np.float64) @ Wv[0:256].astype(np.float64)
           ).astype(f32)
    out = np.zeros((128, 2, 128), f32)
    out[:, 0, :] = Wfc[0:128]
    out[:, 1, :] = Wfc[128:256]
    c['Wfc'] = out
    c['bveffcol'] = (inp['bfeat'].astype(np.float64)
                     @ Wv[0:256].astype(np.float64)
                     + inp['bview'].astype(np.float64)
                     ).astype(f32).reshape(-1, 1)
    perm39 = list(range(3, 39)) + [0, 1, 2]
    c['Wv_app'] = np.ascontiguousarray(Wv_app[perm39])           # [39,128]
    c['Wv_d_lin'] = np.ascontiguousarray(Wv_d[0:3])
    c['Wv_d_sin'] = np.ascontiguousarray(Wv_d[3:27])
    c['Wv_emb'] = np.ascontiguousarray(Wv_emb)
    c['Wv_t_lin'] = np.ascontiguousarray(Wv_t[0:1])
    c['Wv_t_sin'] = np.ascontiguousarray(Wv_t[1:13])
    c['Wsig'] = np.stack([inp['Wsig'][0:128, 0],
                          inp['Wsig'][128:256, 0]], 1).copy()    # [128,2]
    c['Wrgb'] = inp['Wrgb'].copy()                               # [128,3]
    c['brgbcol'] = inp['brgb'].reshape(-1, 1).copy()             # [3,1]
    c['emb_table'] = inp['emb_table'].copy()

    c['sgrid'] = np.broadcast_to(
        np.arange(129, dtype=f32) / 128.0, (128, 129)).copy()
    c['identity'] = np.eye(128, dtype=f32)
    E = np.zeros((4, 512), f32)
    for rl in range(4):
        E[rl, rl * 128:(rl + 1) * 128] = 1.0
    c['Etile'] = E
    c['iotacol'] = np.arange(100, dtype=f32).reshape(-1, 1)
    scalars = dict(pbo_f=float(inp['pbo'][0]), bsig_f=float(inp['bsig'][0]))
    return c, scalars


INPUT_SHAPES = {
    'rays': (R, 12),
    'cA3selT': (3, 60), 'cphasecol': (60, 1),
    'fA3T': (3, 106), 'fA4T': (4, 106),
    'AdT': (4, 24), 'AtT': (2, 12),
    'pW0sin': (60, 128), 'pW0lin': (3, 128),
    'pW1': (128, 128), 'pW2': (128, 128), 'pWo': (128, 1),
    'pb0col': (128, 1), 'pb1col': (128, 1), 'pb2col': (128, 1),
    'fW0my': (63, 256), 'fWm0': (128, 4, 128), 'fWm1': (128, 4, 128),
    'fWm2': (128, 4, 128), 'fWp0': (128, 4, 128), 'fWp1': (128, 4, 128),
    'fWp2': (128, 4, 128), 'fWs_h': (128, 4, 128), 'fWs_e': (63, 256),
    'fb0col': (128, 2), 'fbm0col': (128, 2), 'fbm1col': (128, 2),
    'fbm2col': (128, 2), 'fbp0col': (128, 2), 'fbp1col': (128, 2),
    'fbp2col': (128, 2), 'fbscol': (128, 2),
    'Wfc': (128, 2, 128), 'bveffcol': (128, 1), 'Wv_app': (39, 128),
    'Wv_d_lin': (3, 128), 'Wv_d_sin': (24, 128), 'Wv_emb': (48, 128),
    'Wv_t_lin': (1, 128), 'Wv_t_sin': (12, 128),
    'Wsig': (128, 2), 'Wrgb': (128, 3), 'brgbcol': (3, 1),
    'emb_table': (100, 48),
    'sgrid': (128, 129), 'identity': (128, 128),
    'Etile': (4, 512), 'iotacol': (100, 1),
}
F32R_WEIGHTS = {'fW0my', 'fWm0', 'fWm1', 'fWm2', 'fWp0', 'fWp1', 'fWp2',
                'fWs_h', 'fWs_e', 'Wfc', 'Wv_app', 'Wv_d_lin', 'Wv_d_sin',
                'Wv_emb', 'Wv_t_lin', 'Wv_t_sin', 'Wsig', 'Wrgb',
                'emb_table', 'Etile'}


# ---------------------------------------------------------------- bass build
def build_nc(pbo_f, bsig_f, stage=3, debug=False):
    nc = bacc.Bacc("TRN2", target_bir_lowering=False)
    D = {k: nc.dram_tensor(k, list(v), F32, kind="ExternalInput")
         for k, v in INPUT_SHAPES.items()}
    OUT = nc.dram_tensor("rgb_out", [R, 3], F32, kind="ExternalOutput")
    dbg = {}
    if debug:
        for nm, shp in [("d_sigc", (R, S)), ("d_zf", (R, S + 1)),
                        ("d_wc", (R, S)), ("d_sigf", (R, S)),
                        ("d_wf", (R, S)), ("d_hvray", (128, R)),
                        ("d_ec", (63, CN)), ("d_efa", (63, CN)),
                        ("d_efb", (39, CN)), ("d_h1", (128, 2 * TILE_N))]:
            dbg[nm] = nc.dram_tensor(nm, list(shp), F32, kind="ExternalOutput")
    with tile.TileContext(nc) as tc:
        _body(nc, tc, D, OUT, dbg, pbo_f, bsig_f, stage, debug)
    nc.compile()
    return nc


def _body(nc, tc, D, OUT, dbg, pbo_f, bsig_f, stage, debug):
    from contextlib import ExitStack
    ctx = ExitStack()
    wpool = ctx.enter_context(tc.tile_pool(name="w", bufs=1))
    per = ctx.enter_context(tc.tile_pool(name="per", bufs=1))
    pp2 = ctx.enter_context(tc.tile_pool(name="pp2", bufs=2))
    big = ctx.enter_context(tc.tile_pool(name="big", bufs=2))
    hp = ctx.enter_context(tc.tile_pool(name="h", bufs=3))
    dram = ctx.enter_context(tc.tile_pool(name="dr", bufs=2, space="DRAM"))
    psA = ctx.enter_context(tc.tile_pool(name="psA", bufs=3, space="PSUM"))
    psS = ctx.enter_context(tc.tile_pool(name="psS", bufs=2, space="PSUM"))
    psR = ctx.enter_context(tc.tile_pool(name="psR", bufs=1, space="PSUM"))
    psC = ctx.enter_context(tc.tile_pool(name="psC", bufs=1, space="PSUM"))

    W = {}
    for k, t in D.items():
        if k == 'rays':
            continue
        dt = F32R if k in F32R_WEIGHTS else F32
        tl = wpool.tile(list(t.shape), dt, tag="w_" + k)
        nc.sync.dma_start(tl[:], t[:].bitcast(F32R) if dt == F32R else t[:])
        W[k] = tl
    rays = wpool.tile([R, 12], F32, tag="w_rays")
    nc.sync.dma_start(rays[:], D['rays'][:])
    ident = W['identity']

    # ---------------- phase 0: per-ray prep (ray-major layout)
    nearc = per.tile([R, 1], F32)
    nc.vector.tensor_scalar(nearc[:], rays[:, 6:7], 1e-8, None, op0=OP.max)
    spanc = per.tile([R, 1], F32)
    nc.vector.tensor_tensor(spanc[:], rays[:, 7:8], nearc[:], op=OP.subtract)

    dsq = per.tile([R, 3], F32)
    nc.vector.tensor_tensor(dsq[:], rays[:, 3:6], rays[:, 3:6], op=OP.mult)
    ssum = per.tile([R, 1], F32)
    nc.vector.reduce_sum(ssum[:], dsq[:], axis=mybir.AxisListType.X)
    norm = per.tile([R, 1], F32)
    nc.scalar.activation(norm[:], ssum[:], AF.Sqrt)
    for it in range(2):
        t1 = per.tile([R, 1], F32, tag="nwt")
        nc.vector.reciprocal(t1[:], norm[:])
        nc.vector.scalar_tensor_tensor(t1[:], ssum[:], 1.0, t1[:],
                                       op0=OP.mult, op1=OP.mult)
        nc.vector.tensor_tensor(t1[:], t1[:], norm[:], op=OP.add)
        nc.vector.tensor_scalar(norm[:], t1[:], 0.5, None, op0=OP.mult)
    invn = per.tile([R, 1], F32)
    nc.vector.reciprocal(invn[:], norm[:])

    # bundle: 0:3 oc, 3 ones | 4:7 dc | 8:11 o, 11 ones | 12:15 dir |
    #         16:19 viewdir, 19 ones | 20 t, 21 ones | 22 embid
    bundle = per.tile([R, 28], F32)
    nc.gpsimd.memset(bundle[:], 0.0)
    nc.vector.scalar_tensor_tensor(bundle[:, 0:3], rays[:, 3:6], nearc[:],
                                   rays[:, 0:3], op0=OP.mult, op1=OP.add)
    nc.vector.memset(bundle[:, 3:4], 1.0)
    nc.vector.tensor_scalar(bundle[:, 4:7], rays[:, 3:6], spanc[:], None,
                            op0=OP.mult)
    nc.vector.tensor_copy(bundle[:, 8:11], rays[:, 0:3])
    nc.vector.memset(bundle[:, 11:12], 1.0)
    nc.vector.tensor_copy(bundle[:, 12:15], rays[:, 3:6])
    nc.vector.tensor_scalar(bundle[:, 16:19], rays[:, 3:6], invn[:], None,
                            op0=OP.mult)
    nc.vector.memset(bundle[:, 19:20], 1.0)
    nc.vector.tensor_copy(bundle[:, 20:21], rays[:, 8:9])
    nc.vector.memset(bundle[:, 21:22], 1.0)
    nc.vector.tensor_copy(bundle[:, 22:23], rays[:, 9:10])

    def transp(col):
        p = psC.tile([4, 128], F32, tag="ptp")
        nc.tensor.transpose(p[:], bundle[:, col:col + 4], ident[:])
        sb = per.tile([4, 128], F32, tag="tp%d" % col)
        nc.scalar.copy(sb[:], p[:])
        return sb

    ocT = transp(0)      # [ocT;ones]
    dcT = transp(4)      # [dcT;..]
    oT = transp(8)       # [oT;ones]
    dirT = transp(12)
    vdT = transp(16)     # [viewdirT;ones]
    tT = transp(20)      # [t;ones;embid]
    eiT = transp(22)     # row0 = embid (base 0 for partition_broadcast)

    def mm_copy(lhsT, rhs, shape, nm, dst_dtype=F32):
        p = psC.tile(shape, F32, tag="pmc")
        nc.tensor.matmul(p[:], lhsT, rhs, start=True, stop=True)
        sb = per.tile(shape, dst_dtype, tag="mc_" + nm)
        nc.scalar.copy(sb[:], p[:])
        return sb

    Bf = mm_copy(W['fA3T'][:], dirT[0:3, :], [106, 128], "Bf")
    Cf = mm_copy(W['fA4T'][:], oT[:], [106, 128], "Cf")

    def rangered_v(ap, shape, tag):
        sc = per.tile(shape, F32, tag=tag)
        nc.vector.tensor_scalar(sc[:], ap, float(INV2PI), float(MAGIC),
                                op0=OP.mult, op1=OP.add)
        nc.vector.tensor_scalar(sc[:], sc[:], float(MAGIC), None,
                                op0=OP.subtract)
        nc.vector.scalar_tensor_tensor(ap, sc[:], -float(C1), ap,
                                       op0=OP.mult, op1=OP.add)
        nc.vector.scalar_tensor_tensor(ap, sc[:], -float(C2), ap,
                                       op0=OP.mult, op1=OP.add)

    # per-ray view features
    argd = mm_copy(W['AdT'][:], vdT[:], [24, 128], 'argd')
    rangered_v(argd[:], [24, 128], "rrd")
    sind = per.tile([24, 128], F32R)
    nc.scalar.activation(sind[:], argd[:], AF.Sin)
    vd_r = per.tile([4, 128], F32R)
    nc.vector.tensor_copy(vd_r[:], vdT[:])

    argt = mm_copy(W['AtT'][:], tT[0:2, :], [12, 128], 'argt')
    rangered_v(argt[:], [12, 128], "rrt")
    sint = per.tile([12, 128], F32R)
    nc.scalar.activation(sint[:], argt[:], AF.Sin)
    t_r = per.tile([4, 128], F32R)
    nc.vector.tensor_copy(t_r[:], tT[:])

    embBC = per.tile([100, 128], F32)
    nc.gpsimd.partition_broadcast(embBC[:], eiT[0:1, :], channels=100)
    onehot = per.tile([100, 128], F32R)
    nc.vector.tensor_scalar(onehot[:], embBC[:], W['iotacol'][:], None,
                            op0=OP.is_equal)
    embT = mm_copy(W['emb_table'][:], onehot[:], [48, 128], 'embT', dst_dtype=F32R)

    phv = psC.tile([128, 128], F32, tag="pmc")
    nc.tensor.matmul(phv[:], W['Wv_d_lin'][:], vd_r[0:3, :],
                     start=True, stop=False)
    nc.tensor.matmul(phv[:], W['Wv_d_sin'][:], sind[:], start=False, stop=False)
    nc.tensor.matmul(phv[:], W['Wv_emb'][:], embT[:], start=False, stop=False)
    nc.tensor.matmul(phv[:], W['Wv_t_lin'][:], t_r[0:1, :],
                     start=False, stop=False)
    nc.tensor.matmul(phv[:], W['Wv_t_sin'][:], sint[:], start=False, stop=True)
    hvray = per.tile([128, 128], F32)
    nc.vector.tensor_scalar(hvray[:], phv[:], W['bveffcol'][:], None,
                            op0=OP.add)
    if debug:
        nc.sync.dma_start(dbg["d_hvray"][:], hvray[:])
    phvT = psC.tile([128, 128], F32, tag="pmc")
    nc.tensor.transpose(phvT[:], hvray[:], ident[:])
    hvrayT = per.tile([128, 128], F32R)
    nc.scalar.copy(hvrayT[:], phvT[:])
    hvb = dram.tile([128, 128], F32R, tag="hvb")
    nc.sync.dma_start(hvb[:], hvrayT[:])
    hvre = wpool.tile([4, 32, 128], F32R, tag="hvre")
    nc.sync.dma_start(hvre[:], hvb[:].rearrange("(t rl) m -> rl t m", rl=4))

    # coarse z edges
    zc = per.tile([R, S + 1], F32)
    nc.vector.tensor_scalar(zc[:], W['sgrid'][:], spanc[:], None, op0=OP.mult)
    nc.vector.tensor_scalar(zc[:], zc[:], nearc[:], None, op0=OP.add)
    midc = per.tile([R, S], F32)
    nc.vector.tensor_tensor(midc[:], zc[:, 0:S], zc[:, 1:S + 1], op=OP.add)
    nc.vector.tensor_scalar(midc[:], midc[:], 0.5, None, op0=OP.mult)

    # ======================= COARSE PASS =======================
    sigcT = per.tile([R, S], F32, tag="sigcT")
    for ci in range(NCHUNK):
        r0 = ci * CHUNK_RAYS
        mbc = dram.tile([CHUNK_RAYS, S], F32, tag="midb")
        nc.sync.dma_start(mbc[:], midc[r0:r0 + CHUNK_RAYS, :])
        mfc = pp2.tile([1, CN], F32, tag="flat", bufs=1)
        nc.sync.dma_start(mfc[:],
                          mbc[:].rearrange("p f -> (p f)").unsqueeze(0))
        mx3 = pp2.tile([3, CN], F32, tag="mx3", bufs=1)
        nc.gpsimd.partition_broadcast(mx3[:], mfc[:], channels=3)
        xyzc = pp2.tile([3, CN], F32, tag="xyzc", bufs=1)
        d3 = dirT[0:3, r0:r0 + CHUNK_RAYS].unsqueeze(2).broadcast_to(
            [3, CHUNK_RAYS, S])
        o3 = oT[0:3, r0:r0 + CHUNK_RAYS].unsqueeze(2).broadcast_to(
            [3, CHUNK_RAYS, S])
        x3 = xyzc[:].rearrange("p (r s) -> p r s", r=CHUNK_RAYS)
        nc.vector.tensor_tensor(
            x3, mx3[:].rearrange("p (r s) -> p r s", r=CHUNK_RAYS),
            d3, op=OP.mult)
        nc.vector.tensor_tensor(x3, x3, o3, op=OP.add)

        ec = big.tile([60, CN], F32, tag="arg")
        for t in range(NTILE):
            colsq = slice(t * TILE_N, (t + 1) * TILE_N)
            pa = psA.tile([60, TILE_N], F32, tag="mmps", name="pa")
            nc.tensor.matmul(pa[:], W['cA3selT'][:], xyzc[:, colsq],
                             start=True, stop=True)
            nc.vector.tensor_scalar(ec[:, colsq], pa[:], W['cphasecol'][:],
                                    None, op0=OP.add)
        sc = big.tile([96, CN], F32, tag="mbcrr", bufs=1)
        nc.gpsimd.tensor_scalar(sc[0:60, :], ec[0:60, :], float(INV2PI),
                                float(MAGIC), op0=OP.mult, op1=OP.add)
        nc.gpsimd.tensor_scalar(sc[0:60, :], sc[0:60, :], float(MAGIC), None,
                                op0=OP.subtract)
        nc.vector.scalar_tensor_tensor(ec[0:60, :], sc[0:60, :], -float(C1),
                                       ec[0:60, :], op0=OP.mult, op1=OP.add)
        nc.vector.scalar_tensor_tensor(ec[0:60, :], sc[0:60, :], -float(C2),
                                       ec[0:60, :], op0=OP.mult, op1=OP.add)
        nc.scalar.activation(ec[0:60, :], ec[0:60, :], AF.Sin)

        sb_ = dram.tile([1, CN], F32, tag="sigb")
        sigflat = pp2.tile([1, CN], F32, tag="sigflat", bufs=1)
        for t in range(NTILE):
            cols = slice(t * TILE_N, (t + 1) * TILE_N)
            p1 = psA.tile([128, TILE_N], F32, tag="mmps")
            nc.tensor.matmul(p1[:], W['pW0sin'][:], ec[:, cols],
                             start=True, stop=False)
            nc.tensor.matmul(p1[:], W['pW0lin'][:], xyzc[:, cols],
                             start=False, stop=True)
            h1 = hp.tile([128, TILE_N], F32, tag="ch", bufs=2)
            nc.scalar.activation(h1[:], p1[:], AF.Relu, bias=W['pb0col'][:])
            p2 = psA.tile([128, TILE_N], F32, tag="mmps")
            nc.tensor.matmul(p2[:], W['pW1'][:], h1[:], start=True, stop=True)
            h2 = hp.tile([128, TILE_N], F32, tag="ch", bufs=2)
            nc.vector.tensor_scalar(h2[:], p2[:], W['pb1col'][:], 0.0,
                                    op0=OP.add, op1=OP.max)
            p3 = psA.tile([128, TILE_N], F32, tag="mmps")
            nc.tensor.matmul(p3[:], W['pW2'][:], h2[:], start=True, stop=True)
            h3 = hp.tile([128, TILE_N], F32, tag="ch", bufs=2)
            nc.scalar.activation(h3[:], p3[:], AF.Relu, bias=W['pb2col'][:])
            ps_ = psS.tile([1, TILE_N], F32, tag="sigps")
            nc.tensor.matmul(ps_[:], W['pWo'][:], h3[:], start=True, stop=True)
            if t % 2 == 0:
                nc.scalar.copy(sigflat[0:1, cols], ps_[:])
            else:
                nc.vector.tensor_copy(sigflat[0:1, cols], ps_[:])
        nc.sync.dma_start(sb_[:], sigflat[:])
        nc.sync.dma_start(sigcT[r0:r0 + CHUNK_RAYS, :],
                          sb_[:].rearrange("a (p f) -> (a p) f", p=CHUNK_RAYS))

    if debug:
        nc.sync.dma_start(dbg["d_sigc"][:], sigcT[:])
    if stage < 2:
        ctx.close()
        return

    # ======================= raw2weights helper =======================
    def raw2w(sigT_ap, z_lo, z_hi, norm_ap, bias_f, nrows, tag):
        """w = alpha * exclusive-cumprod(1-alpha+1e-10); returns (w, dz)."""
        P = nrows
        dz = per.tile([P, S], F32, tag=tag + "dz")
        nc.vector.tensor_tensor(dz[:], z_hi, z_lo, op=OP.subtract)
        di = per.tile([P, S], F32, tag=tag + "di")
        nc.vector.tensor_scalar(di[:], dz[:], norm_ap, None, op0=OP.mult)
        s1 = per.tile([P, S], F32, tag=tag + "s1")
        nc.vector.tensor_scalar(s1[:], sigT_ap, bias_f, 0.0,
                                op0=OP.add, op1=OP.max)
        ea = per.tile([P, S], F32, tag=tag + "ea")
        nc.vector.tensor_tensor(ea[:], s1[:], di[:], op=OP.mult)
        e = per.tile([P, S], F32, tag=tag + "e")
        nc.scalar.activation(e[:], ea[:], AF.Exp, scale=-1.0)
        al = per.tile([P, S], F32, tag=tag + "al")
        nc.vector.tensor_scalar(al[:], e[:], -1.0, 1.0, op0=OP.mult, op1=OP.add)
        om = per.tile([P, S], F32, tag=tag + "om")
        nc.vector.tensor_scalar(om[:], e[:], 1e-10, None, op0=OP.add)
        tr = per.tile([P, S], F32, tag=tag + "tr")
        nc.vector.tensor_tensor_scan(tr[:], om[:], om[:], 1.0,
                                     op0=OP.mult, op1=OP.bypass)
        w = per.tile([P, S], F32, tag=tag + "w")
        nc.vector.tensor_copy(w[:, 0:1], al[:, 0:1])
        nc.vector.tensor_tensor(w[:, 1:S], al[:, 1:S], tr[:, 0:S - 1],
                                op=OP.mult)
        return w, dz

    zf = per.tile([R, S + 1], F32)
    wc_dbg = []
    for hi in range(2):
        h0 = hi * 64
        hs = slice(h0, h0 + 64)
        wc, dzc = raw2w(sigcT[hs, :], zc[hs, 0:S], zc[hs, 1:S + 1],
                        norm[hs, :], pbo_f, 64, "c%d" % hi)
        wc_dbg.append(wc)
        Wt = per.tile([64, S], F32, tag="Wt%d" % hi)
        nc.vector.tensor_scalar(Wt[:], wc[:], 1e-5, None, op0=OP.add)
        Sx = per.tile([64, S], F32, tag="Sx%d" % hi)
        nc.vector.memset(Sx[:, 0:1], 0.0)
        nc.vector.tensor_tensor_scan(Sx[:, 1:S], Wt[:, 0:S - 1],
                                     Wt[:, 0:S - 1], 0.0,
                                     op0=OP.add, op1=OP.bypass)
        Tt = per.tile([64, 1], F32, tag="Tt%d" % hi)
        nc.vector.tensor_tensor(Tt[:], Sx[:, S - 1:S], Wt[:, S - 1:S],
                                op=OP.add)
        P2 = per.tile([64, S], F32, tag="P2%d" % hi)
        nc.vector.reciprocal(P2[:], Wt[:])
        nc.vector.tensor_tensor(P2[:], P2[:], dzc[:], op=OP.mult)
        Sn = Sx
        nc.vector.tensor_scalar(Sn[:], Sx[:], -1.0, None, op0=OP.mult)
        UT = per.tile([64, S + 1], F32, tag="UT%d" % hi)
        nc.vector.tensor_scalar(UT[:], W['sgrid'][0:64, :], Tt[:], None,
                                op0=OP.mult)
        for j in range(S + 1):
            x_ = pp2.tile([64, S], F32, tag="pdfx%d" % hi, name="x_")
            nc.vector.scalar_tensor_tensor(x_[:], Sn[:], UT[:, j:j + 1],
                                           P2[:], op0=OP.add, op1=OP.mult)
            sc_ = pp2.tile([64, S], F32, tag="pdfsc%d" % hi, name="sc_")
            nc.vector.scalar_tensor_tensor(sc_[:], x_[:], 0.0, dzc[:],
                                           op0=OP.max, op1=OP.min,
                                           accum_out=zf[hs, j:j + 1])
    nc.vector.tensor_scalar(zf[:], zf[:], zc[:, 0:1], None, op0=OP.add)
    if debug:
        nc.sync.dma_start(dbg["d_zf"][:], zf[:])
        nc.sync.dma_start(dbg["d_wc"][0:64, :], wc_dbg[0][:])
        nc.sync.dma_start(dbg["d_wc"][64:128, :], wc_dbg[1][:])
    if stage < 3:
        ctx.close()
        return

    midf = per.tile([R, S], F32)
    nc.vector.tensor_tensor(midf[:], zf[:, 0:S], zf[:, 1:S + 1], op=OP.add)
    nc.vector.tensor_scalar(midf[:], midf[:], 0.5, None, op0=OP.mult)

    # ======================= FINE PASS =======================
    rgbmT = per.tile([3, 128], F32)
    nc.vector.memset(rgbmT[:], 0.0)

    for ci in range(NCHUNK):
        r0 = ci * CHUNK_RAYS
        mb = dram.tile([CHUNK_RAYS, S], F32, tag="midb")
        nc.sync.dma_start(mb[:], midf[r0:r0 + CHUNK_RAYS, :])
        mflat = pp2.tile([1, CN], F32, tag="flat", bufs=1)
        nc.sync.dma_start(mflat[:],
                          mb[:].rearrange("p f -> (p f)").unsqueeze(0))
        mBC = big.tile([106, CN], F32, tag="mbcrr", bufs=1)
        nc.gpsimd.partition_broadcast(mBC[:], mflat[:], channels=106)

        argf = big.tile([106, CN], F32, tag="arg")
        b3 = Bf[:, r0:r0 + CHUNK_RAYS].unsqueeze(2).broadcast_to(
            [106, CHUNK_RAYS, S])
        c3 = Cf[:, r0:r0 + CHUNK_RAYS].unsqueeze(2).broadcast_to(
            [106, CHUNK_RAYS, S])
        a3 = argf[:].rearrange("p (r s) -> p r s", r=CHUNK_RAYS)
        m3 = mBC[:].rearrange("p (r s) -> p r s", r=CHUNK_RAYS)
        nc.vector.tensor_tensor(a3, m3, b3, op=OP.mult)
        nc.gpsimd.tensor_tensor(a3, a3, c3, op=OP.add)
        sc = mBC
        TWOPI = float(np.float32(2.0 * np.pi))
        for lo, hi in ((0, 60), (64, 100)):
            nc.gpsimd.tensor_scalar(sc[lo:hi, :], argf[lo:hi, :], float(INV2PI),
                                    float(MAGIC), op0=OP.mult, op1=OP.add)
            nc.gpsimd.tensor_scalar(sc[lo:hi, :], sc[lo:hi, :], float(MAGIC),
                                    None, op0=OP.subtract)
            nc.vector.scalar_tensor_tensor(argf[lo:hi, :], sc[lo:hi, :],
                                           -TWOPI, argf[lo:hi, :],
                                           op0=OP.mult, op1=OP.add)
        efa = big.tile([63, CN], F32R, tag="efa")
        efb = big.tile([39, CN], F32R, tag="efb")
        nc.scalar.activation(efa[0:60, :], argf[0:60, :], AF.Sin)
        nc.scalar.activation(efb[0:36, :], argf[64:100, :], AF.Sin)
        nc.sync.dma_start(efa[60:63, :], argf[100:103, :].bitcast(F32R))
        nc.sync.dma_start(efb[36:39, :], argf[103:106, :].bitcast(F32R))
        if debug and ci == 0:
            nc.sync.dma_start(dbg["d_efa"][:], efa[:].bitcast(F32))
            nc.sync.dma_start(dbg["d_efb"][:], efb[:].bitcast(F32))

        rgbS = big.tile([3, CN], F32, tag="rgbS")
        sb_ = dram.tile([1, CN], F32, tag="sigb")
        sigflat = pp2.tile([1, CN], F32, tag="sigflat", bufs=1)
        for t in range(NTILE):
            cols = slice(t * TILE_N, (t + 1) * TILE_N)
            gtile = ci * NTILE + t

            pm = [psA.tile([128, TILE_N], F32, tag="mmps", name="pm%d" % _m)
                  for _m in range(2)]
            for m in range(2):
                nc.tensor.matmul(pm[m][:], W['fW0my'][:, m * 128:(m + 1) * 128],
                                 efa[:, cols], start=True, stop=True)
            h = hp.tile([128, 2 * TILE_N], F32R, tag="fh")
            nc.scalar.activation(h[:, 0:TILE_N], pm[0][:], AF.Relu,
                                 bias=W['fb0col'][:, 0:1])
            nc.vector.tensor_scalar(h[:, TILE_N:], pm[1][:],
                                    W['fb0col'][:, 1:2], 0.0,
                                    op0=OP.add, op1=OP.max)
            if debug and ci == 0 and t == 0:
                nc.sync.dma_start(dbg["d_h1"][:], h[:].bitcast(F32))

            def mid_layer(wname, bname, hin, skip=False):
                pmm = [psA.tile([128, TILE_N], F32, tag="mmps",
                                name="pmm%d" % _m) for _m in range(2)]
                for m in range(2):
                    nc.tensor.matmul(pmm[m][:], W[wname][:, m, :],
                                     hin[:, 0:TILE_N], start=True, stop=False)
                    nc.tensor.matmul(pmm[m][:], W[wname][:, 2 + m, :],
                                     hin[:, TILE_N:],
                                     start=False, stop=not skip)
                    if skip:
                        nc.tensor.matmul(pmm[m][:],
                                         W['fWs_e'][:, m * 128:(m + 1) * 128],
                                         efa[:, cols], start=False, stop=True)
                hout = hp.tile([128, 2 * TILE_N], F32R, tag="fh")
                nc.scalar.activation(hout[:, 0:TILE_N], pmm[0][:], AF.Relu,
                                     bias=W[bname][:, 0:1])
                nc.vector.tensor_scalar(hout[:, TILE_N:], pmm[1][:],
                                        W[bname][:, 1:2], 0.0,
                                        op0=OP.add, op1=OP.max)
                return hout

            h = mid_layer('fWm0', 'fbm0col', h)
            h = mid_layer('fWm1', 'fbm1col', h)
            h = mid_layer('fWm2', 'fbm2col', h)
            h = mid_layer('fWs_h', 'fbscol', h, skip=True)
            h = mid_layer('fWp0', 'fbp0col', h)
            h = mid_layer('fWp1', 'fbp1col', h)
            h = mid_layer('fWp2', 'fbp2col', h)

            ps_ = psS.tile([1, TILE_N], F32, tag="sigps")
            nc.tensor.matmul(ps_[:], W['Wsig'][:, 0:1], h[:, 0:TILE_N],
                             start=True, stop=False)
            nc.tensor.matmul(ps_[:], W['Wsig'][:, 1:2], h[:, TILE_N:],
                             start=False, stop=True)
            if t % 2 == 0:
                nc.scalar.copy(sigflat[0:1, cols], ps_[:])
            else:
                nc.vector.tensor_copy(sigflat[0:1, cols], ps_[:])

            pv = psA.tile([128, TILE_N], F32, tag="mmps")
            nc.tensor.matmul(pv[:], W['Wfc'][:, 0, :], h[:, 0:TILE_N],
                             start=True, stop=False)
            nc.tensor.matmul(pv[:], W['Wfc'][:, 1, :], h[:, TILE_N:],
                             start=False, stop=False)
            nc.tensor.matmul(pv[:], W['Wv_app'][:], efb[:, cols],
                             start=False, stop=False)
            nc.tensor.matmul(pv[:], hvre[:, gtile, :], W['Etile'][:],
                             start=False, stop=True)
            hv = hp.tile([128, TILE_N], F32R, tag="fhv", bufs=2)
            nc.vector.tensor_scalar(hv[:], pv[:], 0.0, None, op0=OP.max)

            prgb = psR.tile([3, TILE_N], F32, tag="rgbps")
            nc.tensor.matmul(prgb[:], W['Wrgb'][:], hv[:],
                             start=True, stop=True)
            nc.scalar.activation(rgbS[0:3, cols], prgb[:],
                                 AF.Sigmoid, bias=W['brgbcol'][:])

        nc.sync.dma_start(sb_[:], sigflat[:])
        sigch = pp2.tile([CHUNK_RAYS, S], F32, tag="sigch")
        nc.sync.dma_start(sigch[:],
                          sb_[:].rearrange("a (p f) -> (a p) f", p=CHUNK_RAYS))
        zfc = pp2.tile([CHUNK_RAYS, S + 1], F32, tag="zfc")
        nc.sync.dma_start(zfc[:], zf[r0:r0 + CHUNK_RAYS, :])
        normc = pp2.tile([CHUNK_RAYS, 1], F32, tag="normc")
        nc.sync.dma_start(normc[:], norm[r0:r0 + CHUNK_RAYS, :])

        wf, _dzf = raw2w(sigch[:], zfc[:, 0:S], zfc[:, 1:S + 1],
                         normc[:], bsig_f, CHUNK_RAYS, "f")
        if debug:
            nc.sync.dma_start(dbg["d_sigf"][r0:r0 + CHUNK_RAYS, :], sigch[:])
            nc.sync.dma_start(dbg["d_wf"][r0:r0 + CHUNK_RAYS, :], wf[:])

        wb = dram.tile([CHUNK_RAYS, S], F32, tag="wb")
        nc.sync.dma_start(wb[:], wf[:])
        wflat = pp2.tile([1, CN], F32, tag="flat", bufs=1)
        nc.sync.dma_start(wflat[:],
                          wb[:].rearrange("p f -> (p f)").unsqueeze(0))
        wBC = big.tile([3, CN], F32, tag="arg")
        nc.gpsimd.partition_broadcast(wBC[:], wflat[:], channels=3)
        nc.vector.tensor_tensor(rgbS[0:3, :], rgbS[0:3, :], wBC[0:3, :],
                                op=OP.mult)
        nc.vector.tensor_reduce(
            rgbmT[0:3, r0:r0 + CHUNK_RAYS],
            rgbS[0:3, :].rearrange("p (r s) -> p r s", r=CHUNK_RAYS),
            axis=mybir.AxisListType.X, op=OP.add)

    # out: transpose [3,128] -> [128,3] via DRAM bounce
    rb = dram.tile([3, 128], F32, tag="rb")
    nc.sync.dma_start(rb[:], rgbmT[:])
    rgbout = per.tile([128, 3], F32)
    nc.sync.dma_start(rgbout[:], rb[:].rearrange("c r -> r c"))
    nc.sync.dma_start(OUT[:], rgbout[:])
    ctx.close()


# ---------------------------------------------------------------- entry
_CACHE = {}


def kernel(**inputs):
    inp = {k: np.asarray(v) for k, v in inputs.items()}
    consts, scal = host_prep(inp)
    key = (BUILD_STAGE, DEBUG_OUT, scal['pbo_f'], scal['bsig_f'])
    if key not in _CACHE:
        _CACHE[key] = build_nc(scal['pbo_f'], scal['bsig_f'],
                               stage=BUILD_STAGE, debug=DEBUG_OUT)
    nc = _CACHE[key]
    rays = np.asarray(inp['rays'], np.float32)
    in_maps = []
    for core in range(NCORES):
        m = {k: np.ascontiguousarray(v, dtype=np.float32)
             for k, v in consts.items()}
        m['rays'] = np.ascontiguousarray(rays[core * R:(core + 1) * R])
        in_maps.append(m)
    res = run_bass_kernel_spmd(nc, in_maps, core_ids=list(range(NCORES)))
    globals()['_LAST_RESULTS'] = res
    return np.concatenate([r['rgb_out'] for r in res.results], 0)
